# revision 44
# baseline (speedup 1.0000x reference)
"""Trainium2 Bass kernel for nn_CandidateFinder (LSH hash-equality KNN).

Reference semantics: q/k binarized (x>0), projected by W [64,8], sign bits
packed into an 8-bit bucket code; for each query, return the first 64 key
indices (ascending) whose code equals the query's code, padded with -1.

Build, per batch, a [256, 64+] table of the first-64 key indices per bucket
(rank = prefix-scan of the code-onehot, local_scatter by masked rank), then
gather per query via one-hot matmuls. Sharding: 8 cores = 4 batches x 2
bucket-halves; host sums the pair and subtracts 1 (tables store j+1).

Pipeline notes (cost-model-driven):
- inputs packed [128, 1024]: one [128,512] binarize covers a whole scan
  half (chunk c of j at partitions 64*(c%2), cols 512*(c//2)).
- k/q on the sync DMA queue (consumers are dep-gated near slice-end;
  a queue's first completion sem lands ~1.7us later, so weights ride
  the gpsimd queue in parallel).
- hash = fp16 W-hi + W-lo matmuls accumulating f32 psum at row bases
  0/32 (M=40 zero-pad on the base-0 chunk defines rows 8:32).
- k-side skips the ACT sign entirely: DVE is_gt gives {0,1} bits, the
  agree matmul computes A' = sum sgn*bit, and the one-hot is
  relu(A' + 1 - popcount(code)) with a per-bucket bias vector. ACT
  runs rk0, rk1 back to back, then the q-side signs/relus.
- rank scan halves run back to back on DVE (scan1 chained off scan0's
  tail column); m1 of half-0 on GPSIMD so the DVE goes straight into
  scan1; idx = onehot*rank - 1 (-1 ignored by local_scatter).
- gathers: per-query-block pairs of matmuls (tab0 start / tab1 stop)
  into 512B psum slots, one open group per 2KB psum region (the four
  regions each open their first block early); strided psum -> packed
  fp16 copies on DVE||ACT; out DMAs on the sync + scalar queues.
"""

import numpy as np
import ml_dtypes

B, L, D, NH = 4, 2048, 64, 8
KMAX = 64
MPAD = 40
TABLE_ELEMS = 1026
HALF = L // 2
NBLK = 16

_cache = {}


def _build_program():
    import concourse.bass as bass
    import concourse.mybir as mybir
    from concourse import bacc, tile
    from contextlib import ExitStack

    dt = mybir.dt
    Alu = mybir.AluOpType
    Act = mybir.ActivationFunctionType

    nc = bacc.Bacc("TRN2", target_bir_lowering=False, debug=False)

    kT_d = nc.declare_dram_parameter("kTp", [128, HALF], dt.bfloat16, isOutput=False)
    qT_d = nc.declare_dram_parameter("qTp", [128, HALF], dt.bfloat16, isOutput=False)
    wpk_d = nc.declare_dram_parameter("wpk", [128, 2 * MPAD], dt.float16, isOutput=False)
    sgnc_d = nc.declare_dram_parameter("sgnc", [128, 128], dt.float16, isOutput=False)
    bias2_d = nc.declare_dram_parameter("bias2", [128, 1], dt.float32, isOutput=False)
    out_d = nc.declare_dram_parameter("out", [128, NBLK * KMAX], dt.float16, isOutput=True)

    with ExitStack() as ctx:
        tc = ctx.enter_context(tile.TileContext(nc))
        sb = ctx.enter_context(tc.tile_pool(name="sb", bufs=1))
        hp = ctx.enter_context(tc.tile_pool(name="hp", bufs=2, space="PSUM"))
        aps = ctx.enter_context(tc.tile_pool(name="aps", bufs=2, space="PSUM"))

        from concourse.tile_rust import add_dep_helper  # noqa: E402

        # ---- input DMAs ----
        kT_sb = sb.tile([128, HALF], dt.bfloat16, tag="kT")
        qT_sb = sb.tile([128, HALF], dt.bfloat16, tag="qT")
        wpk_sb = sb.tile([128, 2 * MPAD], dt.float16, tag="wpk")
        sgnc_sb = sb.tile([128, 128], dt.float16, tag="sgnc")
        nc.sync.dma_start(kT_sb[:, 0:512], kT_d[:, 0:512])
        nc.sync.dma_start(kT_sb[:, 512:1024], kT_d[:, 512:1024])
        nc.sync.dma_start(qT_sb[:, 0:512], qT_d[:, 0:512])
        nc.sync.dma_start(qT_sb[:, 512:1024], qT_d[:, 512:1024])
        bias2_sb = sb.tile([128, 1], dt.float32, tag="bias2")
        nc.gpsimd.dma_start(wpk_sb[:], wpk_d[:])
        nc.gpsimd.dma_start(sgnc_sb[:], sgnc_d[:])
        nc.gpsimd.dma_start(bias2_sb[:], bias2_d[:])

        # scatter values: each partition holds 1..L (fp16 ints <= 2048 exact)
        iota_sb = sb.tile([128, L], dt.float16, tag="iota")
        nc.gpsimd.iota(iota_sb[:], pattern=[[1, L]], base=1, channel_multiplier=0,
                       allow_small_or_imprecise_dtypes=True)

        bias7 = sb.tile([128, 1], dt.float32, tag="bias7")
        nc.gpsimd.memset(bias7[:], -7.0)

        # ---- scheduler seeds + spacers. The static scheduler elides a DMA
        # consumer's +1717ns completion sem ONLY when the consumer's serial
        # slot on its engine falls past the producer's slice-end; precisely
        # sized memset spacers land each binarize just after its DMA slice
        # (kA~750, kB~1250, qA~1750, qB~2250), a warm matmul seeds PE, and a
        # tiny activation pins the auto-LoadActFuncSet to t~200. ----
        xk = sb.tile([128, HALF], dt.float16, tag="xk")
        xq = sb.tile([128, HALF], dt.float16, tag="xq")
        scr16 = sb.tile([64, 256], dt.float16, tag="scr16")
        nc.vector.memset(scr16[:], 0.0)
        atl_sink = sb.tile([1, 1], dt.float32, tag="atl_sink")
        nc.scalar.activation(atl_sink[:], scr16[0:1, 0:1], Act.Relu)
        wp = hp.tile([128, 256], dt.float32, tag="hp", name="warm0")
        nc.tensor.matmul(wp[:], lhsT=scr16[:, 0:128], rhs=scr16[:],
                         start=True, stop=True)
        spacer = sb.tile([128, 256], dt.float32, tag="spacer")
        nc.vector.memset(spacer[:, 0:160], 0.0)
        nc.vector.tensor_single_scalar(xk[:, 0:512], kT_sb[:, 0:512],
                                       0.0, Alu.is_gt)
        nc.vector.memset(spacer[:, 0:230], 0.0)
        nc.vector.tensor_single_scalar(xk[:, 512:1024], kT_sb[:, 512:1024],
                                       0.0, Alu.is_gt)
        nc.vector.memset(spacer[:, 0:230], 1.0)
        nc.vector.tensor_single_scalar(xq[:, 0:512], qT_sb[:, 0:512],
                                       0.0, Alu.is_gt)
        nc.vector.memset(spacer[:, 0:250], 0.0)
        nc.vector.tensor_single_scalar(xq[:, 512:1024], qT_sb[:, 512:1024],
                                       0.0, Alu.is_gt)

        def hash_pair(x, col, tag, pad):
            # chunk pair (even at partitions 0:64 -> psum rows 0:8, odd at
            # 64:128 -> rows 32:40), hi+lo fp16 accumulation. pad=True uses
            # the M=40 zero-padded weights on the even chunk so rows 8:32
            # are defined for a batched [0:40] sign.
            t = hp.tile([128, 512], dt.float32, tag="hp", name=f"hp_{tag}")
            mms = []
            m0 = MPAD if pad else NH
            for u, (p0, p1, r0, m) in enumerate(
                    ((0, 64, 0, m0), (64, 128, 32, NH))):
                nc.tensor.matmul(t[r0:r0 + m, :], lhsT=wpk_sb[p0:p1, 0:m],
                                 rhs=x[p0:p1, col:col + 512],
                                 start=True, stop=False)
                mms.append(nc.tensor.matmul(
                    t[r0:r0 + m, :], lhsT=wpk_sb[p0:p1, MPAD:MPAD + m],
                    rhs=x[p0:p1, col:col + 512], start=False, stop=True))
            return t, mms

        # ---- k side: PE order interleaves hash chunks with agree matmuls
        # so each stage is gated by exactly its own inputs ----
        oh_k = sb.tile([128, L], dt.float16, tag="ohk")
        rank = sb.tile([128, L], dt.float16, tag="rank")

        def hash_chunk(x, col, t, p0, r0, m):
            nc.tensor.matmul(t[r0:r0 + m, :], lhsT=wpk_sb[p0:p0 + 64, 0:m],
                             rhs=x[p0:p0 + 64, col:col + 512],
                             start=True, stop=False)
            return nc.tensor.matmul(
                t[r0:r0 + m, :], lhsT=wpk_sb[p0:p0 + 64, MPAD:MPAD + m],
                rhs=x[p0:p0 + 64, col:col + 512], start=False, stop=True)

        tk0 = hp.tile([128, 512], dt.float32, tag="hp", name="hp_k0")
        tk1 = hp.tile([128, 512], dt.float32, tag="hp", name="hp_k1")
        hash_chunk(xk, 0, tk0, 0, 0, MPAD)        # c0 (padded rows for bits)
        hash_chunk(xk, 0, tk0, 64, 32, NH)        # c1
        hash_chunk(xk, 512, tk1, 0, 0, MPAD)      # c2
        hash_chunk(xk, 512, tk1, 64, 32, NH)      # c3
        # k-side "signs" as {0,1} bits on DVE (keeps ACT off the k chain);
        # the agree matmul then computes A' = sum sgn*bit and the relu
        # threshold becomes per-bucket: onehot = relu(A' + 1 - popcount(c))
        bk0 = sb.tile([40, 512], dt.float16, tag="bk0")
        bk1 = sb.tile([40, 512], dt.float16, tag="bk1")
        nc.vector.tensor_single_scalar(bk0[:], tk0[0:40, :], 0.0, Alu.is_gt)
        nc.vector.tensor_single_scalar(bk1[:], tk1[0:40, :], 0.0, Alu.is_gt)
        apt0 = aps.tile([128, 1024], dt.float32, tag="agree", name="aptk0")
        nc.tensor.matmul(apt0[:, 0:512], lhsT=sgnc_sb[0:8, :],
                         rhs=bk0[0:8, :], start=True, stop=True)
        nc.tensor.matmul(apt0[:, 512:1024], lhsT=sgnc_sb[32:40, :],
                         rhs=bk0[32:40, :], start=True, stop=True)
        apt1 = aps.tile([128, 1024], dt.float32, tag="agree", name="aptk1")
        nc.tensor.matmul(apt1[:, 0:512], lhsT=sgnc_sb[0:8, :],
                         rhs=bk1[0:8, :], start=True, stop=True)
        ag11 = nc.tensor.matmul(apt1[:, 512:1024], lhsT=sgnc_sb[32:40, :],
                                rhs=bk1[32:40, :], start=True, stop=True)

        nc.scalar.activation(oh_k[:, 0:1024], apt0[:],
                                   Act.Relu, bias=bias2_sb[:])
        rk1 = nc.scalar.activation(oh_k[:, 1024:2048], apt1[:],
                                   Act.Relu, bias=bias2_sb[:])

        # ---- rank scan + masks (DVE), scatters (Pool) ----
        m1 = sb.tile([128, L], dt.float16, tag="m1")
        idx16 = sb.tile([128, L], dt.int16, tag="idx16")
        tabs = []
        idx_insts = []
        nc.vector.tensor_tensor_scan(
            rank[:, 0:HALF], oh_k[:, 0:HALF], oh_k[:, 0:HALF],
            0.0, Alu.add, Alu.bypass,
        )
        nc.vector.tensor_tensor_scan(
            rank[:, HALF:L], oh_k[:, HALF:L], oh_k[:, HALF:L],
            rank[:, HALF - 1:HALF], Alu.add, Alu.bypass,
        )
        # m1 half-0 on GPSIMD so the DVE can run scan1 immediately after
        # scan0; masks and scatters then pipeline behind the scans
        nc.gpsimd.tensor_mul(m1[:, 0:HALF], oh_k[:, 0:HALF], rank[:, 0:HALF])
        nc.vector.tensor_mul(m1[:, HALF:L], oh_k[:, HALF:L], rank[:, HALF:L])
        for si in range(2):
            lo, hi = HALF * si, HALF * (si + 1)
            idx_insts.append(nc.vector.tensor_single_scalar(
                idx16[:, lo:hi], m1[:, lo:hi], 1.0, Alu.subtract
            ))
            tab = sb.tile([128, TABLE_ELEMS], dt.float16, tag=f"table{si}")
            tabs.append(tab)
            nc.gpsimd.local_scatter(
                tab[:], iota_sb[:, lo:hi], idx16[:, lo:hi],
                channels=128, num_elems=TABLE_ELEMS, num_idxs=HALF,
            )

        # ---- q side: paired signs (no urgency), relu on ACT ----
        q1h = sb.tile([128, L], dt.float16, tag="q1h")
        aq = []
        for h in range(2):
            t, mms = hash_pair(xq, 512 * h, f"q{h}", pad=True)
            if h == 0:
                add_dep_helper(mms[0].ins, ag11.ins, sync=False,
                               reason="k agree before q hash on PE")
            s = sb.tile([40, 512], dt.float16, tag=f"sq{h}")
            sg = nc.scalar.activation(s[:], t[0:40, :], Act.Sign)
            if h == 0:
                add_dep_helper(sg.ins, rk1.ins, sync=False,
                               reason="k relu before q sign on ACT")
            apt = aps.tile([128, 1024], dt.float32, tag="agree", name=f"aptq{h}")
            aq.append(apt)
            nc.tensor.matmul(apt[:, 0:512], lhsT=sgnc_sb[0:8, :],
                             rhs=s[0:8, :], start=True, stop=True)
            nc.tensor.matmul(apt[:, 512:1024], lhsT=sgnc_sb[32:40, :],
                             rhs=s[32:40, :], start=True, stop=True)
        nc.scalar.activation(q1h[:, 0:1024], aq[0][:], Act.Relu, bias=bias7[:])
        nc.scalar.activation(q1h[:, 1024:2048], aq[1][:], Act.Relu, bias=bias7[:])

        # ---- gather: per-block pair of matmuls (tab0 start, tab1 stop)
        # into a 512B psum slot per block (zones let pairs interleave) ----
        gA = aps.tile([128, 1024], dt.float32, tag="agree", name="gA")
        gB = aps.tile([128, 1024], dt.float32, tag="agree", name="gB")
        # psum allows ONE open accumulation group per 2KB region; the two
        # [128,1024] tiles hold four regions (4 blocks each). Open each
        # region's first group early (tab0 side), then close + run the
        # remaining pairs region by region once scatter1 lands.
        def gmm(blk, ti, start, stop):
            op = gA if blk < 8 else gB
            cc = 128 * (blk % 8)
            nc.tensor.matmul(op[:, cc:cc + KMAX],
                             lhsT=q1h[:, 128 * blk:128 * (blk + 1)],
                             rhs=tabs[ti][:, 0:KMAX], start=start, stop=stop)

        for rg in range(4):
            gmm(4 * rg, 0, True, False)
        for rg in range(4):
            gmm(4 * rg, 1, False, True)
            for blk in range(4 * rg + 1, 4 * rg + 4):
                gmm(blk, 0, True, False)
                gmm(blk, 1, False, True)

        # ---- out: strided psum -> packed fp16 SBUF (ACT || DVE), then
        # half-DMAs on sync + scalar ----
        HWC = 8 * KMAX
        outA = sb.tile([128, HWC], dt.float16, tag="outA")
        outB = sb.tile([128, HWC], dt.float16, tag="outB")
        gA_v = gA[:].rearrange("p (b s) -> p b s", b=8)
        gB_v = gB[:].rearrange("p (b s) -> p b s", b=8)
        nc.vector.tensor_copy(outA[:].rearrange("p (b s) -> p b s", b=8),
                              gA_v[:, :, 0:KMAX])
        nc.scalar.activation(outB[:].rearrange("p (b s) -> p b s", b=8),
                             gB_v[:, :, 0:KMAX], Act.Copy)
        nc.sync.dma_start(out_d[:, 0:HWC], outA[:])
        nc.scalar.dma_start(out_d[:, HWC:2 * HWC], outB[:])

    nc.compile()
    return nc


def _get_nc():
    if "nc" not in _cache:
        _cache["nc"] = _build_program()
    return _cache["nc"]


def _pack(xT):
    # [D, L] -> [128, L/2]: j-chunk c (512 cols) goes to partitions
    # 64*(c%2):64*(c%2)+64, cols 512*(c//2):...
    x4 = xT.reshape(D, 4, 512)
    out = np.empty((128, HALF), xT.dtype)
    out[0:64, 0:512] = x4[:, 0]
    out[64:128, 0:512] = x4[:, 1]
    out[0:64, 512:1024] = x4[:, 2]
    out[64:128, 512:1024] = x4[:, 3]
    return out


def _make_in_maps(query, key, W):
    query = np.asarray(query, dtype=np.float32)
    key = np.asarray(key, dtype=np.float32)
    W = np.asarray(W, dtype=np.float32)
    qTp = [
        _pack(np.ascontiguousarray(query[b].T).astype(ml_dtypes.bfloat16))
        for b in range(B)
    ]
    kTp = [
        _pack(np.ascontiguousarray(key[b].T).astype(ml_dtypes.bfloat16))
        for b in range(B)
    ]

    wpk = np.zeros((128, 2 * MPAD), np.float16)
    wpk[0:64, 0:NH] = W.astype(np.float16)
    wpk[0:64, MPAD:MPAD + NH] = (W - wpk[0:64, 0:NH].astype(np.float32)).astype(
        np.float16
    )
    wpk[64:128] = wpk[0:64]

    sgnc = []
    for h in range(2):
        cg = 128 * h + np.arange(128)
        bits = ((cg[None, :] >> np.arange(NH)[:, None]) & 1).astype(np.float32)
        pm = (2.0 * bits - 1.0).astype(np.float16)  # [8, 128]
        arr = np.zeros((128, 128), np.float16)
        for base in (0, 32, 64, 96):
            arr[base:base + NH] = pm
        sgnc.append(arr)
    bias2 = []
    for h in range(2):
        cg = 128 * h + np.arange(128)
        pc = np.array([bin(v).count("1") for v in cg], np.float32)
        bias2.append((1.0 - pc).reshape(128, 1).astype(np.float32))
    return [
        {"qTp": qTp[c // 2], "kTp": kTp[c // 2], "wpk": wpk,
         "sgnc": sgnc[c % 2], "bias2": bias2[c % 2]}
        for c in range(2 * B)
    ]


def _combine(results):
    out = np.empty((B, L, KMAX), dtype=np.int64)
    for b in range(B):
        g = np.zeros((L, KMAX), np.float64)
        for h in range(2):
            r = results[2 * b + h]["out"].astype(np.float64)
            g += r.reshape(128, NBLK, KMAX).transpose(1, 0, 2).reshape(L, KMAX)
        out[b] = (g - 1.0).astype(np.int64)
    return out


def _run_spmd(in_maps, **kwargs):
    from concourse.bass_utils import run_bass_kernel_spmd

    return run_bass_kernel_spmd(_get_nc(), in_maps, list(range(2 * B)), **kwargs)


def kernel(query, key, W, head_idx=0, **_unused):
    in_maps = _make_in_maps(query, key, W)
    res = _run_spmd(in_maps)
    return _combine(res.results)
